# revision 37
# baseline (speedup 1.0000x reference)
"""Trainium2 Bass kernel for the isotropic-gaussian differentiable renderer.

Math: for pixel p=(x,y) and gaussian g:
    w[g,p] = op_g * exp(-0.5*((x-ax_g)^2+(y-ay_g)^2)/var_g)
    img[p,c] = (sum_g w[g,p]*col_gc) / (sum_g w[g,p] + n_chunks*EPS)

The isotropic RBF is separable: w = expx * B with
    expx = exp(s*(x-ax)^2),  B = op*exp(s*(y-ay)^2),  s = -0.5/var.
That turns the 268M-element exp into a few matmuls + 2*N*128 exps.

Per 128-gaussian chunk:
  PE (bf16): arg[g, 0:128]=argx(g,u), arg[g,128:256]=argy(g,v) via a K=11
             matmul against fixed rows built from [u^2hi, u^2lo, u, 1]
             (centered coords u=x-64; hi/lo splits keep the
             catastrophically-cancelling quadratic accurate to ~5e-3 in
             bf16 = ~0.5% in the exp, inside the 2e-2 gate, and it mostly
             cancels in num/den anyway).  bf16 streams 1 cycle/column vs
             4 for f32r; x and y share the quadratic rows so K=11.
  ACT      : exp(arg) -> fp16 written into fused per-chunk blocks
             [expx(128) | B(128) | colors(384)]; the y half lands as the
             den block B = op*expy directly (ln(op) is in the argument)
  DVE      : ONE tensor_tensor per chunk fills all 3 color blocks
             col_c*B from the SAME rounded B (fp16 weight rounding
             cancels in num/den).  Colors ride as fp16 PAIRS so every
             operand's last AP dim is stride-1 x2 and the DVE can use its
             2x 16-bit mode; B/out are addressed [c(bcast/str), y/2, 2].
             One ~384-col op replaces three 128-col ops: the per-op
             ~130ns DVE overhead made the 48-op version the pipeline
             pacer (measured 195ns/op issue rate = 9.4us serial).
  PE (fp16): acc[x, (den|r|g|b)*128+y] += block[0:128]^T @ block[128:640]
             (fp32 PSUM accumulate)

Schedule notes (from NTFF traces): the measured window opens at the
framework's const-memsets and closes at the end of a fixed ~8.5us walrus
teardown (a full 253-semaphore file clear split across engines; the
Tensor engine's 51 serial clears at ~139ns each are the long pole) that
starts once the last DMA lands.  So every ns the last out-DMA byte moves
earlier is a ns off the measured time:
  - all PE-critical coefficients ride ONE DMA on the sync queue (first
    queue to go live, trigger ~7.3us); merging the coef slices kills the
    second trigger's ~1us queue + ~1.8us descriptor-fetch serial cost (a
    3-way split across queues landed no earlier: latency, not size,
    dominates).  Colors ride the scalar queue (idle until its first
    exp).
  - the PSUM->SBUF cast is split across Vector and Scalar into two
    SEPARATE SBUF tiles, vector's CAST emitted FIRST: Tile serializes
    the two acc readers regardless, so the first reader feeds sync's
    bigger 82KB transfer and the serialization lands on scalar's
    smaller one.  Each half then triggers its own out-DMA queue.  The
    out transfer is descriptor-bound (~128 rows -> ~1.5us), not
    byte-bound.
  - the PE clock (HAM) runs ~1.2GHz until ~6us of GAPLESS busy, then
    2.4GHz; WHEN it steps is stochastic (+-1.5us run noise).  Warmup
    matmuls during the DMA wait do NOT ramp it (measured: constant
    426ns/warmup for 3us) and contend with the input DMA — net loss.
  - fp8e4+DoubleRow mains are a measured wash: the PE streams ~1 moving
    element/cycle in both modes, and fp8-out DVE runs 1.57x slower.
  - every DVE operand needs a stride-1 LAST AP dim: a stride-0
    (broadcast) last dim halves the DVE rate (661 vs 351ns measured),
    hence the x2-pair color layout.

Sharding: gaussians split 2048/core across 8 cores; every core
accumulates the full 128x128 image; host sums the 8 partials, divides
num/den and reshapes to the reference's [4,3,64,64] tile layout.
"""
import ml_dtypes
import numpy as np

import concourse.bacc as bacc
import concourse.tile as tile
from concourse import mybir
from concourse.bass_utils import run_bass_kernel_spmd

# Problem constants (hardcoded per harness contract)
N_GAUSS = 16384
H = 128
W = 128
FX = 128.0
FY = 128.0
CX = 64.0
CY = 64.0
EPS = 1e-8
N_CORES = 8
G_PER_CORE = N_GAUSS // N_CORES      # 2048
CHUNK = 128                          # gaussians per matmul chunk
N_CHUNKS = G_PER_CORE // CHUNK       # 16
ARG_W = 256                          # per-chunk args: 128 x | 128 y
OUT_W = 512                          # (den|r|g|b)*128 free width of acc
KARG = 11                            # arg-matmul contraction rows

F32 = mybir.dt.float32
F16 = mybir.dt.float16
BF16 = mybir.dt.bfloat16
F8 = mybir.dt.float8e4

BLK = 640                            # fused per-chunk block width in t3


def build_program():
    """One SPMD Bass program; every core runs it on its gaussian slice."""
    nc = bacc.Bacc("TRN2", target_bir_lowering=False, debug=False,
                   num_devices=N_CORES)
    # [11, 2304] bf16 = [fixed rhs rows (256) | coef rows chunks 0..15]
    cr = nc.dram_tensor("cr", [KARG, ARG_W + N_CHUNKS * CHUNK], BF16,
                        kind="ExternalInput")
    # [128, 96] fp16: opc[p, chunk*6 + c*2 + {0,1}] = color c (x2 pair)
    # of gaussian chunk*128+p; the pair layout keeps every DVE operand's
    # last AP dim stride-1 (a stride-0 last dim halves the DVE rate:
    # 661ns vs 351ns measured for the same 384-col tensor_tensor).
    opc = nc.dram_tensor("opc", [128, N_CHUNKS * 6], F16,
                         kind="ExternalInput")
    # partial accumulator: [x, (den|r|g|b)*128+y], fp16
    out = nc.dram_tensor("out", [128, OUT_W], F16, kind="ExternalOutput")

    with tile.TileContext(nc) as tc:
        with tc.tile_pool(name="ins", bufs=1) as ins_pool, \
             tc.tile_pool(name="expp", bufs=1) as exp_pool, \
             tc.tile_pool(name="args", bufs=3, space="PSUM") as arg_pool, \
             tc.tile_pool(name="acc", bufs=1, space="PSUM") as acc_pool, \
             tc.tile_pool(name="outp", bufs=1) as out_pool:

            cr_t = ins_pool.tile([KARG, ARG_W + N_CHUNKS * CHUNK], BF16)
            opc_t = ins_pool.tile([128, N_CHUNKS * 6], F16)
            # one trigger per input: the PE-critical coef block goes
            # FIRST on sync (earliest-live queue); colors ride scalar,
            # whose ACT_TABLE_LOAD overlaps the trigger.  (A 3-way coef
            # split across queues was measured: no earlier landing — the
            # ~1.8us descriptor-fetch latency dominates, not size.)
            nc.sync.dma_start(out=cr_t, in_=cr[:, :])
            nc.scalar.dma_start(out=opc_t, in_=opc[:, :])

            # fused per-chunk block [expx | B | colr*B | colg*B | colb*B]
            # fp16: fp8+DoubleRow was measured a wash (the PE streams ~1
            # moving element/cycle in BOTH modes, and fp8-out DVE runs
            # 1.57x slower), so fp16 keeps the DVE fast.
            t3 = exp_pool.tile([128, N_CHUNKS, BLK], F16)
            acc = acc_pool.tile([128, OUT_W], F32)

            def coef_ap(chunk):
                return cr_t[:, ARG_W + chunk * CHUNK:
                            ARG_W + (chunk + 1) * CHUNK]

            rhs_ap = cr_t[:, 0:ARG_W]

            # narrow leading groups tighten the pipeline front (chunk 0's
            # main matmul waits on a 1-chunk exp instead of a 4-chunk
            # batch); a narrow TRAILING group does the same for the loop
            # tail: chunk 15's color mul waits on a 1-chunk exp, cutting
            # the final dependency chain into the out-copies.
            group_plan = [(0, 1), (1, 1), (2, 2), (4, 4), (8, 4), (12, 3),
                          (15, 1)]
            for g0c, width in group_plan:
                args = arg_pool.tile([128, 4 * ARG_W], F32, tag="args")
                for k in range(width):
                    chunk = g0c + k
                    nc.tensor.matmul(
                        args[:, k * ARG_W:(k + 1) * ARG_W],
                        coef_ap(chunk),
                        rhs_ap,
                        start=True, stop=True,
                    )
                nc.scalar.activation(
                    out=t3[:, g0c:g0c + width, 0:ARG_W],
                    in_=args[:, :width * ARG_W],
                    func=mybir.ActivationFunctionType.Exp,
                )

            for chunk in range(N_CHUNKS):
                # y half of the exp is B = op*expy (ln(op) in the arg);
                # all 3 color blocks multiply the SAME rounded B in one
                # DVE op so num/den rounding cancels.  Accumulator order:
                # [den|r|g|b].
                b_ap = t3[:, chunk, 128:256].rearrange(
                    "p (y2 two) -> p y2 two", two=2).unsqueeze(
                    1).broadcast_to([128, 3, 64, 2])
                col_ap = opc_t[:, chunk * 6:(chunk + 1) * 6].rearrange(
                    "p (c two) -> p c two", two=2).unsqueeze(
                    2).broadcast_to([128, 3, 64, 2])
                out_ap = t3[:, chunk, 256:BLK].rearrange(
                    "p (c y2 two) -> p c y2 two", c=3, two=2)
                nc.vector.tensor_tensor(
                    out=out_ap, in0=b_ap, in1=col_ap,
                    op=mybir.AluOpType.mult,
                )
                nc.tensor.matmul(
                    acc[:, :],
                    t3[:, chunk, 0:128],
                    t3[:, chunk, 128:BLK],
                    start=(chunk == 0), stop=(chunk == N_CHUNKS - 1),
                )

            # fp32 PSUM -> fp16 SBUF into SEPARATE tiles, two queues.
            # Tile serializes the two acc readers whatever we do (DMA
            # cannot read PSUM directly), so the FIRST reader is the
            # vector CAST feeding sync's bigger 82KB transfer — the
            # serialization penalty then lands on scalar's smaller 49KB
            # transfer, off the critical path.
            # split at 208/304: equalizes the two serialized tail
            # branches (sync: cast+trigger+transfer vs scalar:
            # cast+copy+trigger+transfer) under the measured costs
            # (cast ~200+0.91ns/col, copy ~300+1.04ns/col, triggers
            # ~650ns, transfer ~4.7ns/col).
            out_a = out_pool.tile([128, 208], F16)
            out_b = out_pool.tile([128, OUT_W - 208], F16)
            nc.vector.tensor_copy(out_b[:, :], acc[:, 208:])
            nc.scalar.copy(out=out_a[:, :], in_=acc[:, :208])
            nc.sync.dma_start(out=out[:, 208:], in_=out_b[:, :])
            nc.scalar.dma_start(out=out[:, :208], in_=out_a[:, :])

    nc.compile()
    return nc


_PROGRAM = None


def _get_program():
    global _PROGRAM
    if _PROGRAM is None:
        _PROGRAM = build_program()
    return _PROGRAM


def _quat2mat(q):
    q = q / np.linalg.norm(q)
    w, x, y, z = q
    return np.array([
        [1 - 2 * (y * y + z * z), 2 * (x * y - z * w), 2 * (x * z + y * w)],
        [2 * (x * y + z * w), 1 - 2 * (x * x + z * z), 2 * (y * z - x * w)],
        [2 * (x * z - y * w), 2 * (y * z + x * w), 1 - 2 * (x * x + y * y)],
    ])


def _bf16(x):
    return np.asarray(x, dtype=np.float64).astype(ml_dtypes.bfloat16)


def _hilo_bf16(x):
    """Split f64 x into bf16 hi+lo with hi+lo ~= x to ~2^-16 relative."""
    hi = _bf16(x)
    lo = _bf16(np.asarray(x, dtype=np.float64) - hi.astype(np.float64))
    return hi, lo


def kernel(positions, colors, opacities, scales, qvec, tvec, tile_hw,
           chunk_gauss, _trace=False):
    positions = np.asarray(positions, dtype=np.float32)
    colors = np.asarray(colors, dtype=np.float32)
    opacities = np.asarray(opacities, dtype=np.float32)
    scales = np.asarray(scales, dtype=np.float32)
    qvec = np.asarray(qvec, dtype=np.float32)
    tvec = np.asarray(tvec, dtype=np.float32)
    tile_hw = int(tile_hw)
    chunk_gauss = int(chunk_gauss)
    n = positions.shape[0]
    assert n == N_GAUSS, f"expected {N_GAUSS} gaussians, got {n}"

    # ---- O(N) per-gaussian prep in float64 (rounds to the same f32 values
    # the reference computes, to well within the exp's own error budget) ----
    R = _quat2mat(qvec.astype(np.float64))
    cam = positions.astype(np.float64) @ R.T + tvec.astype(np.float64)
    ax = cam[:, 0] / cam[:, 2] * FX + CX          # [N] screen x center
    ay = cam[:, 1] / cam[:, 2] * FY + CY          # [N] screen y center
    var = scales[:, 0].astype(np.float64) ** 2
    s = -0.5 / var                                # [N] negative inv 2*var

    # centered coords keep the quadratic-expansion terms small (|u|<=64)
    dx = ax - CX
    dy = ay - CY
    op64 = opacities[:, 0].astype(np.float64)

    # K=11 stationary rows per gaussian, for
    #   arg_x = s*u^2 + bx*u + cx            (u = x - 64, bx = -2 s dx)
    #   arg_y = s*v^2 + by*v + cy + ln(op)   (v = y - 64, cx = s dx^2)
    # The u^2 base row is split u2hi+u2lo (both bf16-exact, u2hi top 8
    # bits); s, bx, by, cx, cy are hi/lo split so every bf16 product in
    # the PE is exact and the residual after cancellation is ~5e-3.
    # x and y share the s rows since the quadratic base [u2|v2] is the
    # same function of the column index.
    s_hi, s_lo = _hilo_bf16(s)
    bx_hi, bx_lo = _hilo_bf16(-2.0 * s * dx)
    by_hi, by_lo = _hilo_bf16(-2.0 * s * dy)
    cx_hi, cx_lo = _hilo_bf16(s * dx * dx)
    cy_hi, cy_lo = _hilo_bf16(s * dy * dy + np.log(op64))
    s1 = _bf16(s)
    coef_full = np.stack([
        s_hi, s_lo, s1,
        bx_hi, bx_lo, by_hi, by_lo,
        cx_hi, cx_lo, cy_hi, cy_lo,
    ])                                            # [11, N] bf16

    u = np.arange(W, dtype=np.float64) - CX       # [-64 .. 63]
    u2 = u * u
    u2hi = _bf16(u2).astype(np.float64)           # top 8 bits, exact split
    u2lo = u2 - u2hi                              # integer <= 8, bf16-exact
    zeros = np.zeros(128)
    ones = np.ones(128)

    def row(xpart, ypart):
        return np.concatenate([xpart, ypart])

    rhs_rows = np.stack([
        row(u2hi, u2hi),   # s_hi
        row(u2hi, u2hi),   # s_lo
        row(u2lo, u2lo),   # s1
        row(u, zeros),     # bx_hi
        row(u, zeros),     # bx_lo
        row(zeros, u),     # by_hi
        row(zeros, u),     # by_lo
        row(ones, zeros),  # cx_hi
        row(ones, zeros),  # cx_lo
        row(zeros, ones),  # cy_hi
        row(zeros, ones),  # cy_lo
    ])                                            # [11, 256]
    rhs_bf16 = _bf16(rhs_rows)

    # ---- shard gaussians across the 8 cores ----
    in_maps = []
    for core in range(N_CORES):
        g0 = core * G_PER_CORE
        g1 = g0 + G_PER_CORE
        coef_core = coef_full[:, g0:g1]           # [11, 2048] bf16
        cr_core = np.concatenate([rhs_bf16, coef_core], axis=1)
        # colors as fp16 pairs: [128, chunk*6 + c*2 + {0,1}]
        col_c = colors[g0:g1].astype(np.float16).reshape(
            N_CHUNKS, CHUNK, 3)
        opc_c = np.ascontiguousarray(np.repeat(
            col_c.transpose(1, 0, 2).reshape(CHUNK, N_CHUNKS * 3), 2,
            axis=1))
        in_maps.append({
            "cr": np.ascontiguousarray(cr_core),
            "opc": opc_c,
        })

    nc = _get_program()
    res = run_bass_kernel_spmd(nc, in_maps, list(range(N_CORES)),
                               trace=_trace)

    # ---- host reduction: sum per-core partials, divide, reshape ----
    acc = np.zeros((128, 4, 128), dtype=np.float64)   # [x, (den|r|g|b), y]
    for core in range(N_CORES):
        acc += res.results[core]["out"].astype(np.float64).reshape(
            128, 4, 128)

    num = acc[:, 1:4, :]                          # [x, c, y]
    n_chunks_ref = n // chunk_gauss
    den = acc[:, 0, :] + n_chunks_ref * EPS       # [x, y]
    img = num / den[:, None, :]                   # [x, c, y]
    img = img.transpose(2, 0, 1).reshape(H * W, 3)  # [p=(y,x), c]

    step = tile_hw * tile_hw
    t = (H * W) // step
    out = img.reshape(t, step, 3).transpose(0, 2, 1).reshape(
        t, 3, tile_hw, tile_hw)
    result = out.astype(np.float32)
    if _trace:
        return result, res
    return result


# revision 42
# speedup vs baseline: 1.0442x; 1.0442x over previous
"""Trainium2 Bass kernel for the isotropic-gaussian differentiable renderer.

Math: for pixel p=(x,y) and gaussian g:
    w[g,p] = op_g * exp(-0.5*((x-ax_g)^2+(y-ay_g)^2)/var_g)
    img[p,c] = (sum_g w[g,p]*col_gc) / (sum_g w[g,p] + n_chunks*EPS)

The isotropic RBF is separable: w = expx * B with
    expx = exp(s*(x-ax)^2),  B = op*exp(s*(y-ay)^2),  s = -0.5/var.
That turns the 268M-element exp into a few matmuls + 2*N*128 exps.

Per 128-gaussian chunk:
  PE (bf16): arg[g, 0:128]=argx(g,u), arg[g,128:256]=argy(g,v) via a K=11
             matmul against fixed rows built from [u^2hi, u^2lo, u, 1]
             (centered coords u=x-64; hi/lo splits keep the
             catastrophically-cancelling quadratic accurate to ~5e-3 in
             bf16 = ~0.5% in the exp, inside the 2e-2 gate, and it mostly
             cancels in num/den anyway).  bf16 streams 1 cycle/column vs
             4 for f32r; x and y share the quadratic rows so K=11.
  ACT      : exp(arg) -> fp16 written into fused per-chunk blocks
             [expx(128) | B(128) | colors(384)]; the y half lands as the
             den block B = op*expy directly (ln(op) is in the argument)
  DVE      : ONE tensor_tensor per chunk fills all 3 color blocks
             col_c*B from the SAME rounded B (fp16 weight rounding
             cancels in num/den).  Colors ride as fp16 PAIRS so every
             operand's last AP dim is stride-1 x2 and the DVE can use its
             2x 16-bit mode; B/out are addressed [c(bcast/str), y/2, 2].
             One ~384-col op replaces three 128-col ops: the per-op
             ~130ns DVE overhead made the 48-op version the pipeline
             pacer (measured 195ns/op issue rate = 9.4us serial).
  PE (fp16): acc[x, (den|r|g|b)*128+y] += block[0:128]^T @ block[128:640]
             (fp32 PSUM accumulate)

Schedule notes (from NTFF traces): the measured window opens at the
framework's const-memsets and closes at the end of a fixed ~8.5us walrus
teardown (a full 253-semaphore file clear split across engines; the
Tensor engine's 51 serial clears at ~139ns each are the long pole) that
starts once the last DMA lands.  So every ns the last out-DMA byte moves
earlier is a ns off the measured time:
  - all PE-critical coefficients ride ONE DMA on the sync queue (first
    queue to go live, trigger ~7.3us); merging the coef slices kills the
    second trigger's ~1us queue + ~1.8us descriptor-fetch serial cost (a
    3-way split across queues landed no earlier: latency, not size,
    dominates).  Colors ride the scalar queue (idle until its first
    exp).
  - the PSUM->SBUF cast is split across Vector and Scalar into two
    SEPARATE SBUF tiles, vector's CAST emitted FIRST: Tile serializes
    the two acc readers regardless, so the first reader feeds sync's
    bigger 82KB transfer and the serialization lands on scalar's
    smaller one.  Each half then triggers its own out-DMA queue.  The
    out transfer is descriptor-bound (~128 rows -> ~1.5us), not
    byte-bound.
  - the PE clock (HAM) runs ~1.2GHz until ~6us of GAPLESS busy, then
    2.4GHz; WHEN it steps is stochastic (+-1.5us run noise).  Warmup
    matmuls during the DMA wait do NOT ramp it (measured: constant
    426ns/warmup for 3us) and contend with the input DMA — net loss.
  - fp8e4+DoubleRow mains are a measured wash: the PE streams ~1 moving
    element/cycle in both modes, and fp8-out DVE runs 1.57x slower.
  - every DVE operand needs a stride-1 LAST AP dim: a stride-0
    (broadcast) last dim halves the DVE rate (661 vs 351ns measured),
    hence the x2-pair color layout.

Sharding: gaussians split 2048/core across 8 cores; every core
accumulates the full 128x128 image; host sums the 8 partials, divides
num/den and reshapes to the reference's [4,3,64,64] tile layout.
"""
import ml_dtypes
import numpy as np

import concourse.bacc as bacc
import concourse.tile as tile
from concourse import mybir
from concourse.bass_utils import run_bass_kernel_spmd

# Problem constants (hardcoded per harness contract)
N_GAUSS = 16384
H = 128
W = 128
FX = 128.0
FY = 128.0
CX = 64.0
CY = 64.0
EPS = 1e-8
N_CORES = 8
G_PER_CORE = N_GAUSS // N_CORES      # 2048
CHUNK = 128                          # gaussians per matmul chunk
N_CHUNKS = G_PER_CORE // CHUNK       # 16
ARG_W = 256                          # per-chunk args: 128 x | 128 y
OUT_W = 512                          # (den|r|g|b)*128 free width of acc
KARG = 11                            # arg-matmul contraction rows

F32 = mybir.dt.float32
F16 = mybir.dt.float16
BF16 = mybir.dt.bfloat16
F8 = mybir.dt.float8e4

BLK = 640                            # fused per-chunk block width in t3


def build_program():
    """One SPMD Bass program; every core runs it on its gaussian slice."""
    nc = bacc.Bacc("TRN2", target_bir_lowering=False, debug=False,
                   num_devices=N_CORES)
    # [11, 2304] bf16 = [fixed rhs rows (256) | coef rows chunks 0..15]
    cr = nc.dram_tensor("cr", [KARG, ARG_W + N_CHUNKS * CHUNK], BF16,
                        kind="ExternalInput")
    # [128, 96] fp16: opc[p, chunk*6 + c*2 + {0,1}] = color c (x2 pair)
    # of gaussian chunk*128+p; the pair layout keeps every DVE operand's
    # last AP dim stride-1 (a stride-0 last dim halves the DVE rate:
    # 661ns vs 351ns measured for the same 384-col tensor_tensor).
    opc = nc.dram_tensor("opc", [128, N_CHUNKS * 6], F16,
                         kind="ExternalInput")
    # partial accumulator: [x, (den|r|g|b)*128+y], fp16
    out = nc.dram_tensor("out", [128, OUT_W], F16, kind="ExternalOutput")

    with tile.TileContext(nc) as tc:
        with tc.tile_pool(name="ins", bufs=1) as ins_pool, \
             tc.tile_pool(name="expp", bufs=1) as exp_pool, \
             tc.tile_pool(name="args", bufs=3, space="PSUM") as arg_pool, \
             tc.tile_pool(name="acc", bufs=1, space="PSUM") as acc_pool, \
             tc.tile_pool(name="outp", bufs=1) as out_pool:

            cr_t = ins_pool.tile([KARG, ARG_W + N_CHUNKS * CHUNK], BF16)
            opc_t = ins_pool.tile([128, N_CHUNKS * 6], F16)
            # one trigger per input: the PE-critical coef block goes
            # FIRST on sync (earliest-live queue); colors ride scalar,
            # whose ACT_TABLE_LOAD overlaps the trigger.  (A 3-way coef
            # split across queues was measured: no earlier landing — the
            # ~1.8us descriptor-fetch latency dominates, not size.)
            nc.sync.dma_start(out=cr_t, in_=cr[:, :])
            nc.scalar.dma_start(out=opc_t, in_=opc[:, :])

            # fused per-chunk block [expx | B | colr*B | colg*B | colb*B]
            # fp16: fp8+DoubleRow was measured a wash (the PE streams ~1
            # moving element/cycle in BOTH modes, and fp8-out DVE runs
            # 1.57x slower), so fp16 keeps the DVE fast.
            t3 = exp_pool.tile([128, N_CHUNKS, BLK], F16)
            acc = acc_pool.tile([128, OUT_W], F32)

            def coef_ap(chunk):
                return cr_t[:, ARG_W + chunk * CHUNK:
                            ARG_W + (chunk + 1) * CHUNK]

            rhs_ap = cr_t[:, 0:ARG_W]

            # narrow leading groups tighten the pipeline front (chunk 0's
            # main matmul waits on a 1-chunk exp instead of a 4-chunk
            # batch); a narrow TRAILING group does the same for the loop
            # tail: chunk 15's color mul waits on a 1-chunk exp, cutting
            # the final dependency chain into the out-copies.
            group_plan = [(0, 1), (1, 1), (2, 2), (4, 4), (8, 4), (12, 3),
                          (15, 1)]
            for g0c, width in group_plan:
                args = arg_pool.tile([128, 4 * ARG_W], F32, tag="args")
                for k in range(width):
                    chunk = g0c + k
                    nc.tensor.matmul(
                        args[:, k * ARG_W:(k + 1) * ARG_W],
                        coef_ap(chunk),
                        rhs_ap,
                        start=True, stop=True,
                    )
                nc.scalar.activation(
                    out=t3[:, g0c:g0c + width, 0:ARG_W],
                    in_=args[:, :width * ARG_W],
                    func=mybir.ActivationFunctionType.Exp,
                )

            for chunk in range(N_CHUNKS):
                # y half of the exp is B = op*expy (ln(op) in the arg);
                # all 3 color blocks multiply the SAME rounded B in one
                # DVE op so num/den rounding cancels.  Accumulator order:
                # [den|r|g|b].
                b_ap = t3[:, chunk, 128:256].rearrange(
                    "p (y2 two) -> p y2 two", two=2).unsqueeze(
                    1).broadcast_to([128, 3, 64, 2])
                col_ap = opc_t[:, chunk * 6:(chunk + 1) * 6].rearrange(
                    "p (c two) -> p c two", two=2).unsqueeze(
                    2).broadcast_to([128, 3, 64, 2])
                out_ap = t3[:, chunk, 256:BLK].rearrange(
                    "p (c y2 two) -> p c y2 two", c=3, two=2)
                nc.vector.tensor_tensor(
                    out=out_ap, in0=b_ap, in1=col_ap,
                    op=mybir.AluOpType.mult,
                )
                nc.tensor.matmul(
                    acc[:, :],
                    t3[:, chunk, 0:128],
                    t3[:, chunk, 128:BLK],
                    start=(chunk == 0), stop=(chunk == N_CHUNKS - 1),
                )

            # fp32 PSUM -> fp16 SBUF into SEPARATE tiles, two queues.
            # Tile serializes the two acc readers whatever we do (DMA
            # cannot read PSUM directly), so the FIRST reader is the
            # vector CAST feeding sync's bigger 82KB transfer — the
            # serialization penalty then lands on scalar's smaller 49KB
            # transfer, off the critical path.
            # split at 208/304: equalizes the two serialized tail
            # branches (sync: cast+trigger+transfer vs scalar:
            # cast+copy+trigger+transfer) under the measured costs
            # (cast ~200+0.91ns/col, copy ~300+1.04ns/col, triggers
            # ~650ns, transfer ~4.7ns/col).
            out_a = out_pool.tile([128, 208], F16)
            out_b = out_pool.tile([128, OUT_W - 208], F16)
            nc.vector.tensor_copy(out_b[:, :], acc[:, 208:])
            nc.scalar.copy(out=out_a[:, :], in_=acc[:, :208])
            nc.sync.dma_start(out=out[:, 208:], in_=out_b[:, :])
            nc.scalar.dma_start(out=out[:, :208], in_=out_a[:, :])

    nc.compile()
    return nc


_PROGRAM = None


def _get_program():
    global _PROGRAM
    if _PROGRAM is None:
        _PROGRAM = build_program()
    return _PROGRAM


def _quat2mat(q):
    q = q / np.linalg.norm(q)
    w, x, y, z = q
    return np.array([
        [1 - 2 * (y * y + z * z), 2 * (x * y - z * w), 2 * (x * z + y * w)],
        [2 * (x * y + z * w), 1 - 2 * (x * x + z * z), 2 * (y * z - x * w)],
        [2 * (x * z - y * w), 2 * (y * z + x * w), 1 - 2 * (x * x + y * y)],
    ])


def _bf16(x):
    return np.asarray(x, dtype=np.float64).astype(ml_dtypes.bfloat16)


def _hilo_bf16(x):
    """Split f64 x into bf16 hi+lo with hi+lo ~= x to ~2^-16 relative."""
    hi = _bf16(x)
    lo = _bf16(np.asarray(x, dtype=np.float64) - hi.astype(np.float64))
    return hi, lo


def kernel(positions, colors, opacities, scales, qvec, tvec, tile_hw,
           chunk_gauss, _trace=False):
    positions = np.asarray(positions, dtype=np.float32)
    colors = np.asarray(colors, dtype=np.float32)
    opacities = np.asarray(opacities, dtype=np.float32)
    scales = np.asarray(scales, dtype=np.float32)
    qvec = np.asarray(qvec, dtype=np.float32)
    tvec = np.asarray(tvec, dtype=np.float32)
    tile_hw = int(tile_hw)
    chunk_gauss = int(chunk_gauss)
    n = positions.shape[0]
    assert n == N_GAUSS, f"expected {N_GAUSS} gaussians, got {n}"

    # ---- O(N) per-gaussian prep in float64 (rounds to the same f32 values
    # the reference computes, to well within the exp's own error budget) ----
    R = _quat2mat(qvec.astype(np.float64))
    cam = positions.astype(np.float64) @ R.T + tvec.astype(np.float64)
    ax = cam[:, 0] / cam[:, 2] * FX + CX          # [N] screen x center
    ay = cam[:, 1] / cam[:, 2] * FY + CY          # [N] screen y center
    var = scales[:, 0].astype(np.float64) ** 2
    s = -0.5 / var                                # [N] negative inv 2*var

    # centered coords keep the quadratic-expansion terms small (|u|<=64)
    dx = ax - CX
    dy = ay - CY
    op64 = opacities[:, 0].astype(np.float64)

    # K=11 stationary rows per gaussian, for
    #   arg_x = s*u^2 + bx*u + cx            (u = x - 64, bx = -2 s dx)
    #   arg_y = s*v^2 + by*v + cy + ln(op)   (v = y - 64, cx = s dx^2)
    # The u^2 base row is split u2hi+u2lo (both bf16-exact, u2hi top 8
    # bits); s, bx, by, cx, cy are hi/lo split so every bf16 product in
    # the PE is exact and the residual after cancellation is ~5e-3.
    # x and y share the s rows since the quadratic base [u2|v2] is the
    # same function of the column index.
    s_hi, s_lo = _hilo_bf16(s)
    bx_hi, bx_lo = _hilo_bf16(-2.0 * s * dx)
    by_hi, by_lo = _hilo_bf16(-2.0 * s * dy)
    cx_hi, cx_lo = _hilo_bf16(s * dx * dx)
    cy_hi, cy_lo = _hilo_bf16(s * dy * dy + np.log(op64))
    s1 = _bf16(s)
    coef_full = np.stack([
        s_hi, s_lo, s1,
        bx_hi, bx_lo, by_hi, by_lo,
        cx_hi, cx_lo, cy_hi, cy_lo,
    ])                                            # [11, N] bf16

    u = np.arange(W, dtype=np.float64) - CX       # [-64 .. 63]
    u2 = u * u
    u2hi = _bf16(u2).astype(np.float64)           # top 8 bits, exact split
    u2lo = u2 - u2hi                              # integer <= 8, bf16-exact
    zeros = np.zeros(128)
    ones = np.ones(128)

    def row(xpart, ypart):
        return np.concatenate([xpart, ypart])

    rhs_rows = np.stack([
        row(u2hi, u2hi),   # s_hi
        row(u2hi, u2hi),   # s_lo
        row(u2lo, u2lo),   # s1
        row(u, zeros),     # bx_hi
        row(u, zeros),     # bx_lo
        row(zeros, u),     # by_hi
        row(zeros, u),     # by_lo
        row(ones, zeros),  # cx_hi
        row(ones, zeros),  # cx_lo
        row(zeros, ones),  # cy_hi
        row(zeros, ones),  # cy_lo
    ])                                            # [11, 256]
    rhs_bf16 = _bf16(rhs_rows)

    # ---- shard gaussians across the 8 cores ----
    in_maps = []
    for core in range(N_CORES):
        g0 = core * G_PER_CORE
        g1 = g0 + G_PER_CORE
        coef_core = coef_full[:, g0:g1]           # [11, 2048] bf16
        cr_core = np.concatenate([rhs_bf16, coef_core], axis=1)
        # colors as fp16 pairs: [128, chunk*6 + c*2 + {0,1}]
        col_c = colors[g0:g1].astype(np.float16).reshape(
            N_CHUNKS, CHUNK, 3)
        opc_c = np.ascontiguousarray(np.repeat(
            col_c.transpose(1, 0, 2).reshape(CHUNK, N_CHUNKS * 3), 2,
            axis=1))
        in_maps.append({
            "cr": np.ascontiguousarray(cr_core),
            "opc": opc_c,
        })

    nc = _get_program()
    res = run_bass_kernel_spmd(nc, in_maps, list(range(N_CORES)),
                               trace=_trace)

    # ---- host reduction: sum per-core partials, divide, reshape ----
    acc = np.zeros((128, 4, 128), dtype=np.float64)   # [x, (den|r|g|b), y]
    for core in range(N_CORES):
        acc += res.results[core]["out"].astype(np.float64).reshape(
            128, 4, 128)

    num = acc[:, 1:4, :]                          # [x, c, y]
    n_chunks_ref = n // chunk_gauss
    den = acc[:, 0, :] + n_chunks_ref * EPS       # [x, y]
    img = num / den[:, None, :]                   # [x, c, y]
    img = img.transpose(2, 0, 1).reshape(H * W, 3)  # [p=(y,x), c]

    step = tile_hw * tile_hw
    t = (H * W) // step
    out = img.reshape(t, step, 3).transpose(0, 2, 1).reshape(
        t, 3, tile_hw, tile_hw)
    result = out.astype(np.float32)
    if _trace:
        return result, res
    return result
